# revision 1
# baseline (speedup 1.0000x reference)
"""Trainium2 Bass kernel for nn_Bihomogeneous_k3.

Math (per batch row, complex z of dim 5 given as z_re/z_im):
  zz[m]   = z_i z_j z_k for the 35 triples i<=j<=k (lexicographic)
  prod    = zz[p] * conj(zz[q]) for the 630 pairs p<=q (lexicographic)
  out     = [Re(prod) (630) | Im(prod) on strict pairs p<q (595)]   -> [B, 1225]

Distribution: pure data parallel over 8 NeuronCores (batch sharded).

Per-core design (B_local = 16384 rows):
  Layout: batch-major megatiles [128 partitions, G=32 groups, features],
  row b = mt*4096 + p*32 + g. All f32.
  - zz stage: complex mults via c-packed tensor_tensor ops with broadcast
    (step-0) and reversed (negative-step) access patterns; on DVE + GPSIMD.
  - pair products, per p-block (re: Rp*R[p:] + Ip*I[p:]; im strict:
    Ip*R[p+1:] - Rp*I[p+1:], via a negated-im copy of zz):
      * DVE writes one product half straight into PSUM,
      * the other half goes to SBUF (DVE or GPSIMD, greedy-balanced),
      * ONE identity-weight fp32 matmul per 512-elem piece accumulates it
        onto the PSUM half (start=False; exact for fp32; PSUM has_written
        bits are primed at kernel start so first-touch accumulates too),
      * ScalarE (ACT) drains PSUM -> SBUF output chunks; HWDGE DMAs out.
  - tiny blocks (w<=8) skip PSUM: mult + add on DVE/GPSIMD directly.
Cost-model estimate ~343us/core vs ~230us HBM-write roofline; engines are
balanced within ~15% of each other (DVE/GPS/PE/DMA/ACT).
"""
import sys

sys.path.insert(0, "/opt/trn_rl_repo")

import numpy as np

N = 5
NC = 8
B_FULL = 131072
B_LOCAL = B_FULL // NC
P = 128
G = 64
ROWS_PER_MT = P * G  # 4096
N_MT = B_LOCAL // ROWS_PER_MT  # 4

# ---- index tables (python-time constants) ----
TRIPLES = [(i, j, k) for i in range(N) for j in range(N) for k in range(N) if i <= j <= k]
M = len(TRIPLES)  # 35
WPAIRS = [(i, j) for i in range(N) for j in range(i, N)]  # 15, lex order
WOFF = {}
_o = 0
for (i, j) in WPAIRS:
    WOFF[(i, j)] = _o
    _o += 1
# zz offsets: triples are (i,j) pairs each followed by k=j..4 (lex order)
ZOFF = {}
_o = 0
for (i, j) in WPAIRS:
    ZOFF[(i, j)] = _o
    _o += N - j
assert _o == M

# re block p covers output cols [REOFF[p], REOFF[p]+35-p); im after 630
REOFF = np.concatenate([[0], np.cumsum([M - p for p in range(M)])]).astype(int)
IMOFF = np.concatenate([[0], np.cumsum([M - 1 - p for p in range(M - 1)])]).astype(int)
N_RE = int(REOFF[M])        # 630
N_IM = int(IMOFF[M - 1])    # 595
N_OUT = N_RE + N_IM         # 1225


# output column chunks (block-aligned). Each entry: (colbase, cols, blocks)
# where blocks is a list of ("re"/"im", p, block_col_base_in_chunk)
def _make_chunks(max_cols=448):
    blocks = []
    for p in range(M):
        blocks.append(("re", p, int(REOFF[p]), M - p))
    for p in range(M - 1):
        blocks.append(("im", p, N_RE + int(IMOFF[p]), M - 1 - p))
    chunks = []
    cur = []
    base = 0
    cols = 0
    for kind, p, cb, w in blocks:
        if cols + w > max_cols and cur:
            chunks.append((base, cols, cur))
            base = cb
            cols = 0
            cur = []
        cur.append((kind, p, cb - base, w))
        cols += w
    if cur:
        chunks.append((base, cols, cur))
    return chunks



def _ap(base_ap, offset_elems, dims, bassmod):
    """Build a raw AP from a tile's base AP: dims = [[step, count], ...] in
    elements, offset_elems added to the base offset."""
    return bassmod.AP(tensor=base_ap.tensor, offset=base_ap.offset + offset_elems,
                      ap=[list(base_ap.ap[0])] + [list(d) for d in dims])


def build_bass(n_mt=N_MT, g=G):
    import concourse.bacc as bacc
    import concourse.bass as bass
    import concourse.tile as tile
    from concourse import mybir
    from contextlib import ExitStack

    f32 = mybir.dt.float32
    b_local = P * g * n_mt

    nc = bacc.Bacc(None)
    z_re_d = nc.dram_tensor("z_re", [b_local, N], f32, kind="ExternalInput")
    z_im_d = nc.dram_tensor("z_im", [b_local, N], f32, kind="ExternalInput")
    ident_d = nc.dram_tensor("ident", [P, P], f32, kind="ExternalInput")
    out_d = nc.dram_tensor("out", [b_local, N_OUT], f32, kind="ExternalOutput")

    chunk_cols = 448 if g <= 32 else 160
    chunks = _make_chunks(chunk_cols)
    piece_w = max(4, 1024 // g)

    # greedy DVE/GPS load balancing (units: ns, cost-model calibrated)
    # DVE sbuf op ~ 121 + 1.042*fd ; DVE psum-dest op ~ 250 + 1.042*fd
    # GPS op ~ 95 + 1.984*fd (fp32 TT at 0.42 efficiency)
    eng_load = {"v": 0.0, "g": 0.0}

    def pick(fd):
        cv = eng_load["v"] + 121 + 1.042 * fd
        cg = eng_load["g"] + 95 + 1.984 * fd
        if cv <= cg:
            eng_load["v"] = cv
            return nc.vector
        eng_load["g"] = cg
        return nc.gpsimd

    with tile.TileContext(nc) as tc:
        with ExitStack() as ctx:
            const_pool = ctx.enter_context(tc.tile_pool(name="const", bufs=1))
            zpool = ctx.enter_context(tc.tile_pool(name="zp", bufs=2))
            wpool = ctx.enter_context(tc.tile_pool(name="wp", bufs=2))
            zzpool = ctx.enter_context(tc.tile_pool(name="zzp", bufs=2))
            t1pool = ctx.enter_context(tc.tile_pool(name="t1p", bufs=2))
            trpool = ctx.enter_context(tc.tile_pool(name="trp", bufs=6))
            outpool = ctx.enter_context(tc.tile_pool(name="outp", bufs=2))
            psum_pool = ctx.enter_context(tc.tile_pool(name="ps", bufs=4, space="PSUM"))

            ident = const_pool.tile([P, P], f32)
            nc.sync.dma_start(out=ident, in_=ident_d[:, :])
            # Warm-up matmul: its only dependency is the ident DMA, so the
            # (single) sync wait fp32 Matmult supports is that DMA. PE is
            # FIFO, so every later matmul sees loaded weights.
            warm = psum_pool.tile([P, 1], f32, tag="ps")
            nc.tensor.matmul(warm, ident, ident[:, 0:1], start=True, stop=True)
            # Prime the PSUM has_written bits over every address the ps-tag
            # slots cover: a start=False matmul only ACCUMULATES where the PE
            # has written since power-on (bit=1); elsewhere it overwrites,
            # which would drop the DVE-written half on first use of a bank.
            identb = ident[:, :]
            for _slot in range(4):
                pt = psum_pool.tile([P, g * piece_w], f32, tag="ps")
                ptb = pt[:, :]
                for half in range(0, g * piece_w, 512):
                    pw = min(512, g * piece_w - half)
                    rhs = _ap(identb, 0, [[0, pw]], bass)
                    pso = _ap(ptb, half, [[1, pw]], bass)
                    nc.tensor.matmul(pso, ident, rhs, start=True, stop=True,
                                     skip_group_check=True)

            mult = mybir.AluOpType.mult
            add = mybir.AluOpType.add
            sub = mybir.AluOpType.subtract

            for mt in range(n_mt):
                r0 = mt * P * g
                # ---- load z: z2 [P, 2, g, N] (c outer) ----
                z2 = zpool.tile([P, 2, g, N], f32)
                src_re = z_re_d[r0:r0 + P * g, :].rearrange("(p g) f -> p g f", g=g)
                src_im = z_im_d[r0:r0 + P * g, :].rearrange("(p g) f -> p g f", g=g)
                nc.sync.dma_start(out=z2[:, 0, :, :], in_=src_re)
                nc.sync.dma_start(out=z2[:, 1, :, :], in_=src_im)
                zb = z2[:, :, :, :]  # base AP; free dims [2*g*N] strides: c=g*N, g=N, f=1
                cZ, gZ = g * N, N

                # ---- w stage: w2 [P, 2, g, 15] ----
                w2 = wpool.tile([P, 2, g, len(WPAIRS)], f32)
                wb = w2[:, :, :, :]
                cW, gW = g * len(WPAIRS), len(WPAIRS)
                for i in range(N):
                    ti_ = N - i
                    off = WOFF[(i, i)]
                    # m1 = (zre_i, zim_i) bcast * (zre[i:], zim[i:]) -> [P, g, 2, ti]
                    t1 = t1pool.tile([P, g, 2, N], f32)
                    t1b = t1[:, :, :, :]
                    in0 = _ap(zb, i, [[gZ, g], [cZ, 2], [0, ti_]], bass)
                    in1 = _ap(zb, i, [[gZ, g], [cZ, 2], [1, ti_]], bass)
                    o1 = _ap(t1b, 0, [[2 * N, g], [N, 2], [1, ti_]], bass)
                    pick(2 * g * ti_).tensor_tensor(out=o1, in0=in0, in1=in1, op=mult)
                    # w_re[i block] = m1[c0] - m1[c1]
                    a0 = _ap(t1b, 0, [[2 * N, g], [1, ti_]], bass)
                    a1 = _ap(t1b, N, [[2 * N, g], [1, ti_]], bass)
                    ow = _ap(wb, off, [[gW, g], [1, ti_]], bass)
                    pick(g * ti_).tensor_tensor(out=ow, in0=a0, in1=a1, op=sub)
                    # m2 = (zim_i, zre_i) bcast * (zre[i:], zim[i:])
                    t2 = t1pool.tile([P, g, 2, N], f32, tag="t2")
                    t2b = t2[:, :, :, :]
                    in0r = _ap(zb, cZ + i, [[gZ, g], [-cZ, 2], [0, ti_]], bass)
                    o2 = _ap(t2b, 0, [[2 * N, g], [N, 2], [1, ti_]], bass)
                    pick(2 * g * ti_).tensor_tensor(out=o2, in0=in0r, in1=in1, op=mult)
                    a0 = _ap(t2b, 0, [[2 * N, g], [1, ti_]], bass)
                    a1 = _ap(t2b, N, [[2 * N, g], [1, ti_]], bass)
                    ow = _ap(wb, cW + off, [[gW, g], [1, ti_]], bass)
                    pick(g * ti_).tensor_tensor(out=ow, in0=a0, in1=a1, op=add)

                # ---- zz stage: zz3 [P, 3, g, 35] (re, im, negim) ----
                zz3 = zzpool.tile([P, 3, g, M], f32)
                zzb = zz3[:, :, :, :]
                cA, gA = g * M, M
                for (i, j) in WPAIRS:
                    tk = N - j
                    pr = WOFF[(i, j)]
                    zo = ZOFF[(i, j)]
                    # m3 = (wre, wim) bcast * (zre[j:], zim[j:])
                    t3 = t1pool.tile([P, g, 2, N], f32, tag="t3")
                    t3b = t3[:, :, :, :]
                    in0 = _ap(wb, pr, [[gW, g], [cW, 2], [0, tk]], bass)
                    in1 = _ap(zb, j, [[gZ, g], [cZ, 2], [1, tk]], bass)
                    o3 = _ap(t3b, 0, [[2 * N, g], [N, 2], [1, tk]], bass)
                    pick(2 * g * tk).tensor_tensor(out=o3, in0=in0, in1=in1, op=mult)
                    a0 = _ap(t3b, 0, [[2 * N, g], [1, tk]], bass)
                    a1 = _ap(t3b, N, [[2 * N, g], [1, tk]], bass)
                    oz = _ap(zzb, zo, [[gA, g], [1, tk]], bass)
                    pick(g * tk).tensor_tensor(out=oz, in0=a0, in1=a1, op=sub)
                    # m4 = (wim, wre) bcast * (zre[j:], zim[j:])
                    t4 = t1pool.tile([P, g, 2, N], f32, tag="t4")
                    t4b = t4[:, :, :, :]
                    in0r = _ap(wb, cW + pr, [[gW, g], [-cW, 2], [0, tk]], bass)
                    o4 = _ap(t4b, 0, [[2 * N, g], [N, 2], [1, tk]], bass)
                    pick(2 * g * tk).tensor_tensor(out=o4, in0=in0r, in1=in1, op=mult)
                    a0 = _ap(t4b, 0, [[2 * N, g], [1, tk]], bass)
                    a1 = _ap(t4b, N, [[2 * N, g], [1, tk]], bass)
                    oz = _ap(zzb, cA + zo, [[gA, g], [1, tk]], bass)
                    pick(g * tk).tensor_tensor(out=oz, in0=a0, in1=a1, op=add)
                # negim slot: zz3[:,2] = -zz3[:,1]
                src = _ap(zzb, cA, [[gA, g], [1, M]], bass)
                dst = _ap(zzb, 2 * cA, [[gA, g], [1, M]], bass)
                eng_load["g"] += 95 + 1.1 * g * M
                nc.gpsimd.tensor_scalar_mul(out=dst, in0=src, scalar1=-1.0)

                # ---- products: DVE half -> PSUM, PE accumulates SBUF half,
                # ---- ACT drains PSUM -> out chunk, chunk DMA'd out
                for (colbase, cols, blist) in chunks:
                    outc = outpool.tile([P, g, chunk_cols], f32)
                    ocb = outc[:, :, :]
                    gO = chunk_cols

                    def prod_aps(kind, p, t0, w):
                        """(inA0, inA1, inB0, inB1) for a block sub-range
                        [t0, t0+w) of block p's products (A half -> PSUM via
                        DVE; B half -> SBUF for the PE accumulate)."""
                        if kind == "re":
                            # A: Rp * R[p+t0:] ; B: Ip * I[p+t0:]
                            return (_ap(zzb, p, [[gA, g], [0, w]], bass),
                                    _ap(zzb, p + t0, [[gA, g], [1, w]], bass),
                                    _ap(zzb, cA + p, [[gA, g], [0, w]], bass),
                                    _ap(zzb, cA + p + t0, [[gA, g], [1, w]], bass))
                        # A: Ip * R[p+1+t0:] ; B: Rp * (-I[p+1+t0:])
                        return (_ap(zzb, cA + p, [[gA, g], [0, w]], bass),
                                _ap(zzb, p + 1 + t0, [[gA, g], [1, w]], bass),
                                _ap(zzb, p, [[gA, g], [0, w]], bass),
                                _ap(zzb, 2 * cA + p + 1 + t0, [[gA, g], [1, w]], bass))

                    for (kind, p, cb, w) in blist:
                        if w <= 8:
                            # small block: pure DVE/GPS, no PSUM round-trip.
                            # one c-packed mult for both halves, then add.
                            tsm = t1pool.tile([P, g, 2, 8], f32, tag="tsm")
                            tsb = tsm[:, :, :, :]
                            iA0, iA1, iB0, iB1 = prod_aps(kind, p, 0, w)
                            in0 = bass.AP(tensor=iA0.tensor, offset=iA0.offset,
                                          ap=[iA0.ap[0], iA0.ap[1],
                                              [iB0.offset - iA0.offset, 2], iA0.ap[2]])
                            in1 = bass.AP(tensor=iA1.tensor, offset=iA1.offset,
                                          ap=[iA1.ap[0], iA1.ap[1],
                                              [iB1.offset - iA1.offset, 2], iA1.ap[2]])
                            om = _ap(tsb, 0, [[16, g], [8, 2], [1, w]], bass)
                            pick(2 * g * w).tensor_tensor(out=om, in0=in0, in1=in1, op=mult)
                            oc = _ap(ocb, cb, [[gO, g], [1, w]], bass)
                            s0 = _ap(tsb, 0, [[16, g], [1, w]], bass)
                            s1 = _ap(tsb, 8, [[16, g], [1, w]], bass)
                            pick(g * w).tensor_tensor(out=oc, in0=s0, in1=s1, op=add)
                            continue
                        # big block: split into <=32-col pieces (2 PSUM banks)
                        t0 = 0
                        while t0 < w:
                            ww = min(piece_w, w - t0)
                            pst = psum_pool.tile([P, g, ww], f32, tag="ps")
                            psb = pst[:, :, :]
                            tr = trpool.tile([P, g, piece_w], f32, tag="tr")
                            trb = tr[:, :, :]
                            inA0, inA1, inB0, inB1 = prod_aps(kind, p, t0, ww)
                            outA = _ap(psb, 0, [[ww, g], [1, ww]], bass)
                            eng_load["v"] += 250 + 1.042 * g * ww
                            nc.vector.tensor_tensor(out=outA, in0=inA0, in1=inA1, op=mult)
                            outB = _ap(trb, 0, [[ww, g], [1, ww]], bass)  # packed flat
                            pick(g * ww).tensor_tensor(out=outB, in0=inB0, in1=inB1, op=mult)
                            # PE: accumulate sbuf half onto psum, 512-elem flat chunks
                            flat = g * ww
                            f0 = 0
                            while f0 < flat:
                                tcw = min(512, flat - f0)
                                rhs = _ap(trb, f0, [[1, tcw]], bass)
                                pso = _ap(psb, f0, [[1, tcw]], bass)
                                nc.tensor.matmul(pso, ident, rhs, start=False, stop=True,
                                                 skip_group_check=True)
                                f0 += tcw
                            # ACT: drain psum piece -> out chunk columns
                            oc = _ap(ocb, cb + t0, [[gO, g], [1, ww]], bass)
                            nc.scalar.copy(out=oc, in_=_ap(psb, 0, [[ww, g], [1, ww]], bass))
                            t0 += ww
                    # DMA chunk out
                    dst = out_d[r0:r0 + P * g, colbase:colbase + cols].rearrange(
                        "(p g) f -> p g f", g=g)
                    nc.sync.dma_start(out=dst, in_=_ap(ocb, 0, [[gO, g], [1, cols]], bass))

    nc.finalize()
    return nc


_CACHED = {}


def _get_nc():
    if "nc" not in _CACHED:
        _CACHED["nc"] = build_bass()
    return _CACHED["nc"]


def kernel(z_re, z_im):
    from concourse.bass_utils import run_bass_kernel_spmd

    z_re = np.ascontiguousarray(np.asarray(z_re, dtype=np.float32))
    z_im = np.ascontiguousarray(np.asarray(z_im, dtype=np.float32))
    assert z_re.shape == (B_FULL, N), z_re.shape

    nc = _get_nc()
    ident = np.eye(P, dtype=np.float32)
    in_maps = []
    for c in range(NC):
        sl = slice(c * B_LOCAL, (c + 1) * B_LOCAL)
        in_maps.append({
            "z_re": np.ascontiguousarray(z_re[sl]),
            "z_im": np.ascontiguousarray(z_im[sl]),
            "ident": ident,
        })
    res = run_bass_kernel_spmd(nc, in_maps, core_ids=list(range(NC)))
    return np.concatenate([res.results[c]["out"] for c in range(NC)], axis=0)



# revision 3
# speedup vs baseline: 1.3315x; 1.3315x over previous
"""Trainium2 Bass kernel for nn_Bihomogeneous_k3 (bf16 diagonal design).

Math (per batch row, complex z of dim 5 given as z_re/z_im):
  zz[m]   = z_i z_j z_k for the 35 triples i<=j<=k (lexicographic)
  prod    = zz[p] * conj(zz[q]) for the 630 pairs p<=q
  out     = [Re(prod) (630) | Im(prod) strict (595)] -> [B, 1225] f32

Design (pure data parallel over 8 cores, B_local = 16384):
  - Everything on-chip is bf16 (threshold 2e-2 >> bf16's ~2e-3), which
    halves the output-DMA bytes and enables the DVE 2x_1p perf mode.
  - Feature-major SBUF layout [128 part, c, feat, g]: the batch-row dim g
    is innermost (stride 1) in EVERY access pattern, so all tensor_tensor
    ops (even ones broadcasting a feature) qualify for DVE 2x mode.
  - Products organized by DIAGONAL d = q-p: out_d[t] = zz[t]*conj(zz[t+d])
    makes both multiplier slices stride-1 (no step-0 broadcast).
    Output columns are diag-ordered on device; the host applies the fixed
    column permutation back to lex order during unshard (pure relabel).
  - Karatsuba 3-mult: m1=R_p R_q, m2=I_p I_q, m3=(I_p-R_p)(R_q+I_q);
    re = m1+m2, im = m3+m1-m2.  SUM/DIF planes precomputed per row.
  - Adds: re diags d < K_GPS summed on GPSIMD via scalar_tensor_tensor
    (straight into the output chunk); all other sums accumulate in PSUM
    via bf16 +/-identity matmuls (1 cycle/row) and drain on ACT (which
    also converts f32 PSUM -> bf16 out chunk).
  - Mults greedily balanced DVE (2x tensor_tensor) vs GPSIMD (STT).
  - Output written bf16 in diag order; host does out[:, PERM].astype(f32).
"""
import sys

sys.path.insert(0, "/opt/trn_rl_repo")

import numpy as np

N = 5
NC = 8
B_FULL = 131072
B_LOCAL = B_FULL // NC
P = 128
G = 64
NMT = B_LOCAL // (P * G)  # 2
M = 35
N_RE = 630
N_IM = 595
N_OUT = 1225
WIN = 32
K_GPS = 12  # re diags 0..K-1 summed on GPSIMD (skip PSUM)

# ---- index tables ----
WPAIRS = [(i, j) for i in range(N) for j in range(i, N)]  # 15 lex
WOFF = {}
_o = 0
for (i, j) in WPAIRS:
    WOFF[(i, j)] = _o
    _o += 1
ZOFF = {}
_o = 0
for (i, j) in WPAIRS:
    ZOFF[(i, j)] = _o
    _o += N - j
assert _o == M

# diag column layout: re diags d=0..34 (w=35-d), then im diags d=1..34
W_D = [M - d for d in range(M)]
DOFF_RE = np.concatenate([[0], np.cumsum(W_D)]).astype(int)          # [36]
assert DOFF_RE[M] == N_RE
DOFF_IM = {}
_o = N_RE
for d in range(1, M):
    DOFF_IM[d] = _o
    _o += M - d
assert _o == N_OUT

CK = int(DOFF_RE[K_GPS])           # first PE-region column
# output chunks: [0, CK) is the GPS region; PE region split ~evenly
_pe_cols = N_OUT - CK
_s1 = CK + (_pe_cols + 2) // 3
_s2 = CK + 2 * ((_pe_cols + 2) // 3)
CHUNKS = [(0, CK), (CK, _s1), (_s1, _s2), (_s2, N_OUT)]
CMAX = max(e - s for s, e in CHUNKS)
assert all((e - s) * 2 >= 512 for s, e in CHUNKS), CHUNKS  # DMA full rate

# windows over the PE region
WINDOWS = []
_a = CK
while _a < N_OUT:
    WINDOWS.append((_a, min(_a + WIN, N_OUT)))
    _a += WIN

# diag runs: list of (kind, d, colstart, width)
DIAG_RUNS = []
for d in range(K_GPS, M):
    DIAG_RUNS.append(("re", d, int(DOFF_RE[d]), M - d))
for d in range(1, M):
    DIAG_RUNS.append(("im", d, int(DOFF_IM[d]), M - d))

# host-side permutation: lex column j <- diag column PERM[j]
PERM = np.zeros(N_OUT, dtype=np.int64)
_c = 0
for p in range(M):
    for q in range(p, M):
        PERM[_c] = DOFF_RE[q - p] + p
        _c += 1
for p in range(M):
    for q in range(p + 1, M):
        PERM[_c] = DOFF_IM[q - p] + p
        _c += 1
assert _c == N_OUT


def _ap(base_ap, offset_elems, dims, bassmod):
    return bassmod.AP(tensor=base_ap.tensor, offset=base_ap.offset + offset_elems,
                      ap=[list(base_ap.ap[0])] + [list(d) for d in dims])


def build_bass():
    import concourse.bacc as bacc
    import concourse.bass as bass
    import concourse.tile as tile
    from concourse import mybir
    from contextlib import ExitStack

    f32 = mybir.dt.float32
    bf16 = mybir.dt.bfloat16
    mult = mybir.AluOpType.mult
    add = mybir.AluOpType.add
    sub = mybir.AluOpType.subtract

    nc = bacc.Bacc(None)
    z_re_d = nc.dram_tensor("z_re", [B_LOCAL, N], f32, kind="ExternalInput")
    z_im_d = nc.dram_tensor("z_im", [B_LOCAL, N], f32, kind="ExternalInput")
    ident_d = nc.dram_tensor("ident", [P, P], f32, kind="ExternalInput")
    out_d = nc.dram_tensor("out", [B_LOCAL, N_OUT], bf16, kind="ExternalOutput")

    # greedy DVE/GPS balancing (ns, cost-model calibrated).
    # DVE bf16 packed 2x: 60 + 0.521*fd ; GPS scalar_tensor_tensor:
    # 156 + 1.389*fd. est[] also tracks PE/ACT/DMA for the debug printout.
    est = {"v": 0.0, "g": 0.0, "pe": 0.0, "act": 0.0, "dma": 0.0}

    def pick_tt(fd, packed=True):
        cv = est["v"] + 60 + (0.521 if packed else 1.042) * fd
        cg = est["g"] + 156 + 1.389 * fd
        if cv <= cg:
            est["v"] = cv
            return "v"
        est["g"] = cg
        return "g"

    with tile.TileContext(nc) as tc:
        with ExitStack() as ctx:
            cpool = ctx.enter_context(tc.tile_pool(name="const", bufs=1))
            zpool = ctx.enter_context(tc.tile_pool(name="zp", bufs=2))
            wpool = ctx.enter_context(tc.tile_pool(name="wp", bufs=2))
            zzpool = ctx.enter_context(tc.tile_pool(name="zzp", bufs=2))
            tpool = ctx.enter_context(tc.tile_pool(name="tp", bufs=2))
            mgpool = ctx.enter_context(tc.tile_pool(name="mgp", bufs=2))
            mwpool = ctx.enter_context(tc.tile_pool(name="mwp", bufs=3))
            outpool = ctx.enter_context(tc.tile_pool(name="outp", bufs=2))
            pspool = ctx.enter_context(tc.tile_pool(name="ps", bufs=2, space="PSUM"))

            identf = cpool.tile([P, P], f32)
            nc.sync.dma_start(out=identf, in_=ident_d[:, :])
            identP = cpool.tile([P, P], bf16)
            identN = cpool.tile([P, P], bf16)
            nc.scalar.copy(out=identP, in_=identf[:, :])
            nc.scalar.mul(out=identN, in_=identf[:, :], mul=-1.0)
            est["act"] += 2 * (185 + 128 * 0.833)
            idP = identP[:, :]
            idN = identN[:, :]

            # PE warm-up: makes the first weight loads depend only on the
            # ident casts; later matmuls hit a warm FIFO.
            warm = pspool.tile([P, WIN * G], f32, tag="ps")
            wb = warm[:, :]
            nc.tensor.matmul(_ap(wb, 0, [[1, 1]], bass), idP,
                             _ap(idP, 0, [[1, 1]], bass),
                             start=True, stop=True, skip_group_check=True)
            nc.tensor.matmul(_ap(wb, 1, [[1, 1]], bass), idN,
                             _ap(idN, 0, [[1, 1]], bass),
                             start=True, stop=True, skip_group_check=True)

            def tt(eng, out, in0, in1, op):
                if eng == "v":
                    nc.vector.tensor_tensor(out=out, in0=in0, in1=in1, op=op)
                else:
                    nc.gpsimd.scalar_tensor_tensor(out=out, in0=in0, scalar=1.0,
                                                   in1=in1, op0=mult, op1=op)

            cZ, cW, cA = N * G, len(WPAIRS) * G, M * G

            for mt in range(NMT):
                r0 = mt * P * G
                # ---- load z into row-major blob, cast+transpose to zT ----
                blob = zpool.tile([P, 2, G, N], f32, tag="blob")
                src_re = z_re_d[r0:r0 + P * G, :].rearrange("(p g) f -> p g f", g=G)
                src_im = z_im_d[r0:r0 + P * G, :].rearrange("(p g) f -> p g f", g=G)
                nc.sync.dma_start(out=blob[:, 0, :, :], in_=src_re)
                nc.sync.dma_start(out=blob[:, 1, :, :], in_=src_im)
                est["dma"] += 2 * 0.46
                zT = zpool.tile([P, 2, N, G], bf16, tag="zT")
                zb = zT[:, :, :, :]
                bb = blob[:, :, :, :]
                nc.scalar.copy(
                    out=_ap(zb, 0, [[cZ, 2], [G, N], [1, G]], bass),
                    in_=_ap(bb, 0, [[G * N, 2], [1, N], [N, G]], bass))
                est["act"] += 185 + 2 * N * G * 0.833

                # ---- w stage: wT [P, 2, 15, G] ----
                wT = wpool.tile([P, 2, len(WPAIRS), G], bf16)
                wbb = wT[:, :, :, :]
                for i in range(N):
                    ti = N - i
                    off = WOFF[(i, i)]
                    t1 = tpool.tile([P, 2, N, G], bf16, tag="ta")
                    t1b = t1[:, :, :, :]
                    # m1[c] = z[c,i] (bcast) * z[c, i:]
                    in0 = _ap(zb, i * G, [[cZ, 2], [0, ti], [1, G]], bass)
                    in1 = _ap(zb, i * G, [[cZ, 2], [G, ti], [1, G]], bass)
                    o1 = _ap(t1b, 0, [[N * G, 2], [G, ti], [1, G]], bass)
                    tt(pick_tt(2 * ti * G), o1, in0, in1, mult)
                    # w_re run = m1[0] - m1[1]
                    a0 = _ap(t1b, 0, [[G, ti], [1, G]], bass)
                    a1 = _ap(t1b, N * G, [[G, ti], [1, G]], bass)
                    ow = _ap(wbb, off * G, [[G, ti], [1, G]], bass)
                    tt(pick_tt(ti * G), ow, a0, a1, sub)
                    # m2[c] = z[1-c,i] (bcast) * z[c, i:]
                    t2 = tpool.tile([P, 2, N, G], bf16, tag="tb")
                    t2b = t2[:, :, :, :]
                    in0r = _ap(zb, cZ + i * G, [[-cZ, 2], [0, ti], [1, G]], bass)
                    o2 = _ap(t2b, 0, [[N * G, 2], [G, ti], [1, G]], bass)
                    tt(pick_tt(2 * ti * G), o2, in0r, in1, mult)
                    a0 = _ap(t2b, 0, [[G, ti], [1, G]], bass)
                    a1 = _ap(t2b, N * G, [[G, ti], [1, G]], bass)
                    ow = _ap(wbb, cW + off * G, [[G, ti], [1, G]], bass)
                    tt(pick_tt(ti * G), ow, a0, a1, add)

                # ---- zz stage: zzT [P, 4, 35, G] planes R,I,SUM,DIF ----
                zzT = zzpool.tile([P, 4, M, G], bf16)
                ab = zzT[:, :, :, :]
                for (i, j) in WPAIRS:
                    tk = N - j
                    pr = WOFF[(i, j)]
                    zo = ZOFF[(i, j)]
                    t3 = tpool.tile([P, 2, N, G], bf16, tag="ta")
                    t3b = t3[:, :, :, :]
                    in0 = _ap(wbb, pr * G, [[cW, 2], [0, tk], [1, G]], bass)
                    in1 = _ap(zb, j * G, [[cZ, 2], [G, tk], [1, G]], bass)
                    o3 = _ap(t3b, 0, [[N * G, 2], [G, tk], [1, G]], bass)
                    tt(pick_tt(2 * tk * G), o3, in0, in1, mult)
                    a0 = _ap(t3b, 0, [[G, tk], [1, G]], bass)
                    a1 = _ap(t3b, N * G, [[G, tk], [1, G]], bass)
                    oz = _ap(ab, zo * G, [[G, tk], [1, G]], bass)
                    tt(pick_tt(tk * G), oz, a0, a1, sub)
                    t4 = tpool.tile([P, 2, N, G], bf16, tag="tb")
                    t4b = t4[:, :, :, :]
                    in0r = _ap(wbb, cW + pr * G, [[-cW, 2], [0, tk], [1, G]], bass)
                    o4 = _ap(t4b, 0, [[N * G, 2], [G, tk], [1, G]], bass)
                    tt(pick_tt(2 * tk * G), o4, in0r, in1, mult)
                    a0 = _ap(t4b, 0, [[G, tk], [1, G]], bass)
                    a1 = _ap(t4b, N * G, [[G, tk], [1, G]], bass)
                    oz = _ap(ab, cA + zo * G, [[G, tk], [1, G]], bass)
                    tt(pick_tt(tk * G), oz, a0, a1, add)
                # SUM = R + I (plane 2), DIF = I - R (plane 3)
                aR = _ap(ab, 0, [[G, M], [1, G]], bass)
                aI = _ap(ab, cA, [[G, M], [1, G]], bass)
                tt(pick_tt(M * G), _ap(ab, 2 * cA, [[G, M], [1, G]], bass), aR, aI, add)
                tt(pick_tt(M * G), _ap(ab, 3 * cA, [[G, M], [1, G]], bass), aI, aR, sub)

                # ---- products ----
                chunk_tiles = {}

                def get_chunk(ci):
                    if ci not in chunk_tiles:
                        occ_t = outpool.tile([P, G, CMAX], bf16, tag="oc",
                                             name=f"oc{mt}_{ci}")
                        chunk_tiles[ci] = occ_t
                    return chunk_tiles[ci]

                # GPS region: diags d < K_GPS, re only, summed on GPSIMD
                oc0 = get_chunk(0)[:, :, :]
                C0 = CHUNKS[0][1]
                for d in range(K_GPS):
                    w = M - d
                    mg = mgpool.tile([P, 2, M, G], bf16, tag="mg")
                    mgb = mg[:, :, :, :]
                    in0 = _ap(ab, 0, [[cA, 2], [G, w], [1, G]], bass)
                    in1 = _ap(ab, d * G, [[cA, 2], [G, w], [1, G]], bass)
                    om = _ap(mgb, 0, [[M * G, 2], [G, w], [1, G]], bass)
                    tt(pick_tt(2 * w * G), om, in0, in1, mult)
                    # re = m1 + m2 straight into the chunk (GPS, any AP ok)
                    og = _ap(oc0, int(DOFF_RE[d]), [[1, w], [CMAX, G]], bass)
                    m1 = _ap(mgb, 0, [[G, w], [1, G]], bass)
                    m2 = _ap(mgb, M * G, [[G, w], [1, G]], bass)
                    nc.gpsimd.scalar_tensor_tensor(out=og, in0=m1, scalar=1.0,
                                                   in1=m2, op0=mult, op1=add)
                    est["g"] += 156 + 1.389 * w * G
                # chunk 0 done -> DMA
                dst = out_d[r0:r0 + P * G, 0:C0].rearrange("(p g) f -> p g f", g=G)
                nc.sync.dma_start(out=dst, in_=_ap(oc0, 0, [[CMAX, G], [1, C0]], bass))
                est["dma"] += P * G * C0 * 2 / 360.0 / 1000.0 * 128

                # PE region: windows of 32 cols
                for (wa, wz) in WINDOWS:
                    wcols = wz - wa
                    mw = mwpool.tile([P, 2, WIN, G], bf16, tag="mw")
                    mwb = mw[:, :, :, :]
                    m3w = mwpool.tile([P, WIN, G], bf16, tag="m3w")
                    m3b = m3w[:, :, :]
                    runs = []
                    for (kind, d, cs, w) in DIAG_RUNS:
                        ra, rb = max(wa, cs), min(wz, cs + w)
                        if ra < rb:
                            runs.append((kind, d, cs, ra, rb))
                    # mults for every run in this window
                    for (kind, d, cs, ra, rb) in runs:
                        t0 = ra - cs
                        wl = rb - ra
                        x0 = ra - wa
                        in0 = _ap(ab, t0 * G, [[cA, 2], [G, wl], [1, G]], bass)
                        in1 = _ap(ab, (t0 + d) * G, [[cA, 2], [G, wl], [1, G]], bass)
                        om = _ap(mwb, x0 * G, [[WIN * G, 2], [G, wl], [1, G]], bass)
                        tt(pick_tt(2 * wl * G), om, in0, in1, mult)
                        if kind == "im":
                            iD = _ap(ab, 3 * cA + t0 * G, [[G, wl], [1, G]], bass)
                            iS = _ap(ab, 2 * cA + (t0 + d) * G, [[G, wl], [1, G]], bass)
                            o3 = _ap(m3b, x0 * G, [[G, wl], [1, G]], bass)
                            tt(pick_tt(wl * G), o3, iD, iS, mult)
                    # PE accumulation into PSUM, 8-col (512 elem) pieces
                    ps = pspool.tile([P, WIN * G], f32, tag="ps")
                    psb = ps[:, :]
                    for (kind, d, cs, ra, rb) in runs:
                        x0, x1 = ra - wa, rb - wa
                        xa = x0
                        while xa < x1:
                            xb = min((xa // 8 + 1) * 8, x1)
                            n = (xb - xa) * G
                            pso = _ap(psb, xa * G, [[1, n]], bass)
                            m1 = _ap(mwb, xa * G, [[1, n]], bass)
                            m2 = _ap(mwb, WIN * G + xa * G, [[1, n]], bass)
                            if kind == "re":
                                nc.tensor.matmul(pso, idP, m1, start=True, stop=False,
                                                 skip_group_check=True)
                                nc.tensor.matmul(pso, idP, m2, start=False, stop=True,
                                                 skip_group_check=True)
                                est["pe"] += 2 * n * 0.4167
                            else:
                                m3 = _ap(m3b, xa * G, [[1, n]], bass)
                                nc.tensor.matmul(pso, idP, m3, start=True, stop=False,
                                                 skip_group_check=True)
                                nc.tensor.matmul(pso, idP, m1, start=False, stop=False,
                                                 skip_group_check=True)
                                nc.tensor.matmul(pso, idN, m2, start=False, stop=True,
                                                 skip_group_check=True)
                                est["pe"] += 3 * n * 0.4167
                            xa = xb
                    # drain window -> chunk tile(s) (ACT converts to bf16)
                    for ci, (cb, ce) in enumerate(CHUNKS):
                        sa, sz = max(wa, cb), min(wz, ce)
                        if sa >= sz:
                            continue
                        n = sz - sa
                        occ = get_chunk(ci)[:, :, :]
                        src = _ap(psb, (sa - wa) * G, [[G, n], [1, G]], bass)
                        dstc = _ap(occ, sa - cb, [[1, n], [CMAX, G]], bass)
                        nc.scalar.copy(out=dstc, in_=src)
                        est["act"] += 185 + n * G * 0.833
                    # chunk finished when this window reaches its end
                    for ci, (cb, ce) in enumerate(CHUNKS[1:], start=1):
                        if wa < ce <= wz or (wz == N_OUT and ce == N_OUT):
                            occ = chunk_tiles[ci][:, :, :]
                            cw = ce - cb
                            dst = out_d[r0:r0 + P * G, cb:ce].rearrange(
                                "(p g) f -> p g f", g=G)
                            nc.sync.dma_start(
                                out=dst, in_=_ap(occ, 0, [[CMAX, G], [1, cw]], bass))
                            est["dma"] += P * G * cw * 2 / 360.0 / 1000.0 * 128

    nc.finalize()
    print(f"[build est us/core] DVE={est['v']/1000:.1f} GPS={est['g']/1000:.1f} "
          f"PE={est['pe']/1000:.1f} ACT={est['act']/1000:.1f} DMA={est['dma']/1000:.1f}")
    return nc


_CACHED = {}


def _get_nc():
    if "nc" not in _CACHED:
        _CACHED["nc"] = build_bass()
    return _CACHED["nc"]


def kernel(z_re, z_im):
    from concourse.bass_utils import run_bass_kernel_spmd

    z_re = np.ascontiguousarray(np.asarray(z_re, dtype=np.float32))
    z_im = np.ascontiguousarray(np.asarray(z_im, dtype=np.float32))
    assert z_re.shape == (B_FULL, N), z_re.shape

    nc = _get_nc()
    ident = np.eye(P, dtype=np.float32)
    in_maps = []
    for c in range(NC):
        sl = slice(c * B_LOCAL, (c + 1) * B_LOCAL)
        in_maps.append({
            "z_re": np.ascontiguousarray(z_re[sl]),
            "z_im": np.ascontiguousarray(z_im[sl]),
            "ident": ident,
        })
    res = run_bass_kernel_spmd(nc, in_maps, core_ids=list(range(NC)))
    parts = [np.asarray(res.results[c]["out"]) for c in range(NC)]
    full = np.concatenate(parts, axis=0).astype(np.float32)
    return full[:, PERM]


# revision 5
# speedup vs baseline: 1.3542x; 1.0170x over previous
"""Trainium2 Bass kernel for nn_Bihomogeneous_k3 (bf16 diagonal design).

Math (per batch row, complex z of dim 5 given as z_re/z_im):
  zz[m]   = z_i z_j z_k for the 35 triples i<=j<=k (lexicographic)
  prod    = zz[p] * conj(zz[q]) for the 630 pairs p<=q
  out     = [Re(prod) (630) | Im(prod) strict (595)] -> [B, 1225] f32

Design (pure data parallel over 8 cores, B_local = 16384):
  - Everything on-chip is bf16 (threshold 2e-2 >> bf16's ~2e-3): halves
    output-DMA bytes and enables the DVE 2x_1p perf mode.
  - Feature-major SBUF layout [128 part, c, feat, g]: the batch-row dim g
    is innermost (stride 1) in EVERY access pattern, so all tensor_tensor
    ops (even feature-broadcast ones) qualify for DVE 2x.
  - Products organized by DIAGONAL d = q-p: out_d[t] = zz[t]*conj(zz[t+d])
    keeps both multiplier slices stride-1.  Karatsuba 3-mult per diag:
    m1=R_t R_{t+d}, m2=I_t I_{t+d}, m3=(I_t-R_t)(R_{t+d}+I_{t+d});
    re = m1+m2, im = m3+m1-m2.  m1/m2 computed ONCE per diag and shared
    by the re and im outputs.
  - Device column order (host permutes back during unshard):
      [re_0..re_{K-1}] [im_1..im_{K-1}] [re_K im_K re_{K+1} im_{K+1} ...]
    Region 1 re sums run on GPSIMD scalar_tensor_tensor (skip PSUM);
    everything after col CK accumulates into PSUM via bf16 +/-identity
    matmuls (1 cycle/row), filling 32-col windows sequentially; ACT
    drains each window (converting f32 PSUM -> bf16 output chunk).
  - Mults greedily balanced DVE (2x tensor_tensor) vs GPSIMD (STT).
  - Output bf16, diag order; host does out[:, PERM].astype(f32).
"""
import sys

sys.path.insert(0, "/opt/trn_rl_repo")

import numpy as np

N = 5
NC = 8
B_FULL = 131072
B_LOCAL = B_FULL // NC
P = 128
G = 64
NMT = B_LOCAL // (P * G)  # 2
M = 35
N_RE = 630
N_IM = 595
N_OUT = 1225
WIN = 32
K_GPS = 12  # re diags 0..K-1 summed on GPSIMD (skip PSUM)

# ---- index tables ----
WPAIRS = [(i, j) for i in range(N) for j in range(i, N)]  # 15 lex
WOFF = {}
_o = 0
for (i, j) in WPAIRS:
    WOFF[(i, j)] = _o
    _o += 1
ZOFF = {}
_o = 0
for (i, j) in WPAIRS:
    ZOFF[(i, j)] = _o
    _o += N - j
assert _o == M

# device column layout (diag order):
#   region1: re_d for d < K_GPS               (GPS adds)
#   region2: im_d for d = 1..K_GPS-1          (PSUM)
#   region3: re_d, im_d interleaved, d >= K   (PSUM)
RE_COL = {}
IM_COL = {}
_c = 0
for d in range(K_GPS):
    RE_COL[d] = _c
    _c += M - d
CK = _c  # start of PSUM region
for d in range(1, K_GPS):
    IM_COL[d] = _c
    _c += M - d
for d in range(K_GPS, M):
    RE_COL[d] = _c
    _c += M - d
    IM_COL[d] = _c
    _c += M - d
assert _c == N_OUT

# psum column stream order: (kind, d, colstart, width), column-contiguous
PSUM_RUNS = []
for d in range(1, K_GPS):
    PSUM_RUNS.append(("im", d, IM_COL[d], M - d))
for d in range(K_GPS, M):
    PSUM_RUNS.append(("re", d, RE_COL[d], M - d))
    PSUM_RUNS.append(("im", d, IM_COL[d], M - d))

# windows over the PSUM region
WINDOWS = []
_a = CK
while _a < N_OUT:
    WINDOWS.append((_a, min(_a + WIN, N_OUT)))
    _a += WIN

# output chunks (>=256 cols each for full-rate DMA)
_pe_cols = N_OUT - CK
_t = (_pe_cols + 2) // 3
CHUNKS = [(0, CK), (CK, CK + _t), (CK + _t, CK + 2 * _t), (CK + 2 * _t, N_OUT)]
CMAX = max(e - s for s, e in CHUNKS)
assert all((e - s) * 2 >= 512 for s, e in CHUNKS), CHUNKS

# host-side permutation: lex column j <- device column PERM[j]
PERM = np.zeros(N_OUT, dtype=np.int64)
_c = 0
for p in range(M):
    for q in range(p, M):
        PERM[_c] = RE_COL[q - p] + p
        _c += 1
for p in range(M):
    for q in range(p + 1, M):
        PERM[_c] = IM_COL[q - p] + p
        _c += 1
assert _c == N_OUT


def _ap(base_ap, offset_elems, dims, bassmod):
    return bassmod.AP(tensor=base_ap.tensor, offset=base_ap.offset + offset_elems,
                      ap=[list(base_ap.ap[0])] + [list(d) for d in dims])


def build_bass():
    import concourse.bacc as bacc
    import concourse.bass as bass
    import concourse.tile as tile
    from concourse import mybir
    from contextlib import ExitStack

    f32 = mybir.dt.float32
    bf16 = mybir.dt.bfloat16
    mult = mybir.AluOpType.mult
    add = mybir.AluOpType.add
    sub = mybir.AluOpType.subtract

    nc = bacc.Bacc(None)
    z_re_d = nc.dram_tensor("z_re", [B_LOCAL, N], f32, kind="ExternalInput")
    z_im_d = nc.dram_tensor("z_im", [B_LOCAL, N], f32, kind="ExternalInput")
    ident_d = nc.dram_tensor("ident", [P, P], f32, kind="ExternalInput")
    out_d = nc.dram_tensor("out", [B_LOCAL, N_OUT], bf16, kind="ExternalOutput")

    # greedy DVE/GPS balancing (ns, cost-model calibrated).
    est = {"v": 0.0, "g": 0.0, "pe": 0.0, "act": 0.0, "dma": 0.0}

    def pick_tt(fd, packed=True):
        cv = est["v"] + 60 + (0.521 if packed else 1.042) * fd
        cg = est["g"] + 156 + 1.389 * fd
        if cv <= cg:
            est["v"] = cv
            return "v"
        est["g"] = cg
        return "g"

    with tile.TileContext(nc) as tc:
        with ExitStack() as ctx:
            cpool = ctx.enter_context(tc.tile_pool(name="const", bufs=1))
            zpool = ctx.enter_context(tc.tile_pool(name="zp", bufs=2))
            wpool = ctx.enter_context(tc.tile_pool(name="wp", bufs=2))
            zzpool = ctx.enter_context(tc.tile_pool(name="zzp", bufs=2))
            tpool = ctx.enter_context(tc.tile_pool(name="tp", bufs=2))
            mgpool = ctx.enter_context(tc.tile_pool(name="mgp", bufs=3))
            outpool = ctx.enter_context(tc.tile_pool(name="outp", bufs=2))
            pspool = ctx.enter_context(tc.tile_pool(name="ps", bufs=2, space="PSUM"))

            identf = cpool.tile([P, P], f32)
            nc.sync.dma_start(out=identf, in_=ident_d[:, :])
            identP = cpool.tile([P, P], bf16)
            identN = cpool.tile([P, P], bf16)
            nc.scalar.copy(out=identP, in_=identf[:, :])
            nc.scalar.mul(out=identN, in_=identf[:, :], mul=-1.0)
            est["act"] += 2 * (185 + 128 * 0.833)
            idP = identP[:, :]
            idN = identN[:, :]

            # PE warm-up so early weight loads only depend on the ident casts
            warm = pspool.tile([P, WIN * G], f32, tag="ps")
            wrm = warm[:, :]
            nc.tensor.matmul(_ap(wrm, 0, [[1, 1]], bass), idP,
                             _ap(idP, 0, [[1, 1]], bass),
                             start=True, stop=True, skip_group_check=True)
            nc.tensor.matmul(_ap(wrm, 1, [[1, 1]], bass), idN,
                             _ap(idN, 0, [[1, 1]], bass),
                             start=True, stop=True, skip_group_check=True)

            def tt(eng, out, in0, in1, op):
                if eng == "v":
                    nc.vector.tensor_tensor(out=out, in0=in0, in1=in1, op=op)
                else:
                    nc.gpsimd.scalar_tensor_tensor(out=out, in0=in0, scalar=1.0,
                                                   in1=in1, op0=mult, op1=op)

            cZ, cW, cA = N * G, len(WPAIRS) * G, M * G

            for mt in range(NMT):
                r0 = mt * P * G
                # ---- load z into row-major blob, cast+transpose to zT ----
                blob = zpool.tile([P, 2, G, N], f32, tag="blob")
                src_re = z_re_d[r0:r0 + P * G, :].rearrange("(p g) f -> p g f", g=G)
                src_im = z_im_d[r0:r0 + P * G, :].rearrange("(p g) f -> p g f", g=G)
                nc.sync.dma_start(out=blob[:, 0, :, :], in_=src_re)
                nc.sync.dma_start(out=blob[:, 1, :, :], in_=src_im)
                est["dma"] += 2 * 0.46
                zT = zpool.tile([P, 2, N, G], bf16, tag="zT")
                zb = zT[:, :, :, :]
                bb = blob[:, :, :, :]
                nc.scalar.copy(
                    out=_ap(zb, 0, [[cZ, 2], [G, N], [1, G]], bass),
                    in_=_ap(bb, 0, [[G * N, 2], [1, N], [N, G]], bass))
                est["act"] += 185 + 2 * N * G * 0.833

                # ---- w stage: wT [P, 2, 15, G] ----
                wT = wpool.tile([P, 2, len(WPAIRS), G], bf16)
                wbb = wT[:, :, :, :]
                for i in range(N):
                    ti = N - i
                    off = WOFF[(i, i)]
                    t1 = tpool.tile([P, 2, N, G], bf16, tag="ta")
                    t1b = t1[:, :, :, :]
                    in0 = _ap(zb, i * G, [[cZ, 2], [0, ti], [1, G]], bass)
                    in1 = _ap(zb, i * G, [[cZ, 2], [G, ti], [1, G]], bass)
                    o1 = _ap(t1b, 0, [[N * G, 2], [G, ti], [1, G]], bass)
                    tt(pick_tt(2 * ti * G), o1, in0, in1, mult)
                    a0 = _ap(t1b, 0, [[G, ti], [1, G]], bass)
                    a1 = _ap(t1b, N * G, [[G, ti], [1, G]], bass)
                    ow = _ap(wbb, off * G, [[G, ti], [1, G]], bass)
                    tt(pick_tt(ti * G), ow, a0, a1, sub)
                    t2 = tpool.tile([P, 2, N, G], bf16, tag="tb")
                    t2b = t2[:, :, :, :]
                    in0r = _ap(zb, cZ + i * G, [[-cZ, 2], [0, ti], [1, G]], bass)
                    o2 = _ap(t2b, 0, [[N * G, 2], [G, ti], [1, G]], bass)
                    tt(pick_tt(2 * ti * G), o2, in0r, in1, mult)
                    a0 = _ap(t2b, 0, [[G, ti], [1, G]], bass)
                    a1 = _ap(t2b, N * G, [[G, ti], [1, G]], bass)
                    ow = _ap(wbb, cW + off * G, [[G, ti], [1, G]], bass)
                    tt(pick_tt(ti * G), ow, a0, a1, add)

                # ---- zz stage: zzT [P, 4, 35, G] planes R,I,SUM,DIF ----
                zzT = zzpool.tile([P, 4, M, G], bf16)
                ab = zzT[:, :, :, :]
                for (i, j) in WPAIRS:
                    tk = N - j
                    pr = WOFF[(i, j)]
                    zo = ZOFF[(i, j)]
                    t3 = tpool.tile([P, 2, N, G], bf16, tag="ta")
                    t3b = t3[:, :, :, :]
                    in0 = _ap(wbb, pr * G, [[cW, 2], [0, tk], [1, G]], bass)
                    in1 = _ap(zb, j * G, [[cZ, 2], [G, tk], [1, G]], bass)
                    o3 = _ap(t3b, 0, [[N * G, 2], [G, tk], [1, G]], bass)
                    tt(pick_tt(2 * tk * G), o3, in0, in1, mult)
                    a0 = _ap(t3b, 0, [[G, tk], [1, G]], bass)
                    a1 = _ap(t3b, N * G, [[G, tk], [1, G]], bass)
                    oz = _ap(ab, zo * G, [[G, tk], [1, G]], bass)
                    tt(pick_tt(tk * G), oz, a0, a1, sub)
                    t4 = tpool.tile([P, 2, N, G], bf16, tag="tb")
                    t4b = t4[:, :, :, :]
                    in0r = _ap(wbb, cW + pr * G, [[-cW, 2], [0, tk], [1, G]], bass)
                    o4 = _ap(t4b, 0, [[N * G, 2], [G, tk], [1, G]], bass)
                    tt(pick_tt(2 * tk * G), o4, in0r, in1, mult)
                    a0 = _ap(t4b, 0, [[G, tk], [1, G]], bass)
                    a1 = _ap(t4b, N * G, [[G, tk], [1, G]], bass)
                    oz = _ap(ab, cA + zo * G, [[G, tk], [1, G]], bass)
                    tt(pick_tt(tk * G), oz, a0, a1, add)
                aR = _ap(ab, 0, [[G, M], [1, G]], bass)
                aI = _ap(ab, cA, [[G, M], [1, G]], bass)
                tt(pick_tt(M * G), _ap(ab, 2 * cA, [[G, M], [1, G]], bass), aR, aI, add)
                tt(pick_tt(M * G), _ap(ab, 3 * cA, [[G, M], [1, G]], bass), aI, aR, sub)

                # ---- products ----
                chunk_tiles = {}

                def get_chunk(ci):
                    if ci not in chunk_tiles:
                        occ_t = outpool.tile([P, G, CMAX], bf16, tag="oc",
                                             name=f"oc{mt}_{ci}")
                        chunk_tiles[ci] = occ_t
                    return chunk_tiles[ci]

                ps_tiles = {}  # window idx -> psum tile

                def get_ps(k):
                    if k not in ps_tiles:
                        ps_t = pspool.tile([P, WIN * G], f32, tag="ps",
                                           name=f"ps{mt}_{k}")
                        ps_tiles[k] = ps_t
                    return ps_tiles[k]

                def drain_upto(col):
                    # drain complete windows strictly below `col`
                    for k, (wa, wz) in enumerate(WINDOWS):
                        if k not in ps_tiles or wz > col:
                            continue
                        psb = ps_tiles.pop(k)[:, :]
                        for ci, (cb, ce) in enumerate(CHUNKS):
                            sa, sz = max(wa, cb), min(wz, ce)
                            if sa >= sz:
                                continue
                            n = sz - sa
                            occ = get_chunk(ci)[:, :, :]
                            src = _ap(psb, (sa - wa) * G, [[G, n], [1, G]], bass)
                            dstc = _ap(occ, sa - cb, [[1, n], [CMAX, G]], bass)
                            nc.scalar.copy(out=dstc, in_=src)
                            est["act"] += 185 + n * G * 0.833
                        # DMA any chunk whose end this window crosses
                        for ci, (cb, ce) in enumerate(CHUNKS):
                            if ci > 0 and wa < ce <= wz:
                                occ = chunk_tiles[ci][:, :, :]
                                cw = ce - cb
                                dst = out_d[r0:r0 + P * G, cb:ce].rearrange(
                                    "(p g) f -> p g f", g=G)
                                nc.sync.dma_start(
                                    out=dst,
                                    in_=_ap(occ, 0, [[CMAX, G], [1, cw]], bass))
                                est["dma"] += P * G * cw * 2 / 360.0 / 1000.0 * 128

                def pe_accumulate(kind, d, cs, w, mgb, m3b):
                    # append run [cs, cs+w) to the psum stream
                    ca = cs
                    while ca < cs + w:
                        k = (ca - CK) // WIN
                        wa = WINDOWS[k][0]
                        # split at 8-col grid (psum bank) and window boundary
                        cb_ = min((ca - CK) // 8 * 8 + 8 + CK, WINDOWS[k][1], cs + w)
                        n = (cb_ - ca) * G
                        psb = get_ps(k)[:, :]
                        pso = _ap(psb, (ca - wa) * G, [[1, n]], bass)
                        t0 = ca - cs
                        m1 = _ap(mgb, t0 * G, [[1, n]], bass)
                        m2 = _ap(mgb, M * G + t0 * G, [[1, n]], bass)
                        if kind == "re":
                            nc.tensor.matmul(pso, idP, m1, start=True, stop=False,
                                             skip_group_check=True)
                            nc.tensor.matmul(pso, idP, m2, start=False, stop=True,
                                             skip_group_check=True)
                            est["pe"] += 2 * n * 0.4167
                        else:
                            m3 = _ap(m3b, t0 * G, [[1, n]], bass)
                            nc.tensor.matmul(pso, idP, m3, start=True, stop=False,
                                             skip_group_check=True)
                            nc.tensor.matmul(pso, idP, m1, start=False, stop=False,
                                             skip_group_check=True)
                            nc.tensor.matmul(pso, idN, m2, start=False, stop=True,
                                             skip_group_check=True)
                            est["pe"] += 3 * n * 0.4167
                        ca = cb_

                C0 = CHUNKS[0][1]
                frontier = CK
                for d in range(M):
                    w = M - d
                    # m1/m2 for the whole diag (shared by re and im)
                    mg = mgpool.tile([P, 2, M, G], bf16, tag="mg")
                    mgb = mg[:, :, :, :]
                    in0 = _ap(ab, 0, [[cA, 2], [G, w], [1, G]], bass)
                    in1 = _ap(ab, d * G, [[cA, 2], [G, w], [1, G]], bass)
                    om = _ap(mgb, 0, [[M * G, 2], [G, w], [1, G]], bass)
                    tt(pick_tt(2 * w * G), om, in0, in1, mult)
                    m3b = None
                    if d >= 1:
                        m3g = mgpool.tile([P, M, G], bf16, tag="m3g")
                        m3b = m3g[:, :, :]
                        iD = _ap(ab, 3 * cA, [[G, w], [1, G]], bass)
                        iS = _ap(ab, 2 * cA + d * G, [[G, w], [1, G]], bass)
                        o3 = _ap(m3b, 0, [[G, w], [1, G]], bass)
                        tt(pick_tt(w * G), o3, iD, iS, mult)
                    if d < K_GPS:
                        # re on GPSIMD straight into chunk 0
                        oc0 = get_chunk(0)[:, :, :]
                        og = _ap(oc0, RE_COL[d], [[1, w], [CMAX, G]], bass)
                        m1 = _ap(mgb, 0, [[G, w], [1, G]], bass)
                        m2 = _ap(mgb, M * G, [[G, w], [1, G]], bass)
                        nc.gpsimd.scalar_tensor_tensor(out=og, in0=m1, scalar=1.0,
                                                       in1=m2, op0=mult, op1=add)
                        est["g"] += 156 + 1.389 * w * G
                        if d >= 1:
                            pe_accumulate("im", d, IM_COL[d], w, mgb, m3b)
                            frontier = IM_COL[d] + w
                    else:
                        pe_accumulate("re", d, RE_COL[d], w, mgb, m3b)
                        pe_accumulate("im", d, IM_COL[d], w, mgb, m3b)
                        frontier = IM_COL[d] + w
                    drain_upto(frontier)
                    if d == K_GPS - 1:
                        # chunk 0 complete -> DMA
                        oc0 = chunk_tiles[0][:, :, :]
                        dst = out_d[r0:r0 + P * G, 0:C0].rearrange(
                            "(p g) f -> p g f", g=G)
                        nc.sync.dma_start(out=dst,
                                          in_=_ap(oc0, 0, [[CMAX, G], [1, C0]], bass))
                        est["dma"] += P * G * C0 * 2 / 360.0 / 1000.0 * 128
                drain_upto(N_OUT + 1)

    nc.finalize()
    print(f"[build est us/core] DVE={est['v']/1000:.1f} GPS={est['g']/1000:.1f} "
          f"PE={est['pe']/1000:.1f} ACT={est['act']/1000:.1f} DMA={est['dma']/1000:.1f}")
    return nc


_CACHED = {}


def _get_nc():
    if "nc" not in _CACHED:
        _CACHED["nc"] = build_bass()
    return _CACHED["nc"]


def kernel(z_re, z_im):
    from concourse.bass_utils import run_bass_kernel_spmd

    z_re = np.ascontiguousarray(np.asarray(z_re, dtype=np.float32))
    z_im = np.ascontiguousarray(np.asarray(z_im, dtype=np.float32))
    assert z_re.shape == (B_FULL, N), z_re.shape

    nc = _get_nc()
    ident = np.eye(P, dtype=np.float32)
    in_maps = []
    for c in range(NC):
        sl = slice(c * B_LOCAL, (c + 1) * B_LOCAL)
        in_maps.append({
            "z_re": np.ascontiguousarray(z_re[sl]),
            "z_im": np.ascontiguousarray(z_im[sl]),
            "ident": ident,
        })
    res = run_bass_kernel_spmd(nc, in_maps, core_ids=list(range(NC)))
    parts = [np.asarray(res.results[c]["out"]) for c in range(NC)]
    full = np.concatenate(parts, axis=0).astype(np.float32)
    return full[:, PERM]
